# revision 1
# baseline (speedup 1.0000x reference)
"""CRF loss kernel for 8x Trainium2 NeuronCores (Bass/Tile). Self-contained.

nn_CRF: loss = mean_b( logZ_b - gold_b ) for a linear-chain CRF with
B=512 sequences, T=512 steps, K=64 tags (START=62, STOP=63).

Strategy:
- Data-parallel over batch: core c takes sequences [64c, 64c+64).
- Device computes the forward algorithm in the exp domain:
      P_t = (E @ P_{t-1}) * F_t,      E = exp(transitions),
  with F_t laid out (tag, seq) and pre-scaled on host:
      F_t = softmax_i(feats[:, t-1, :]) * exp(-chat_t)
  where chat_t = log(sum_i softmax_i * rowmean(E)) estimates the per-step
  log-growth. On the real data this keeps all P magnitudes within e^{+-8}
  over 512 steps, so no on-device renormalization is needed; the host adds
  the exactly-known scale factors back in fp64.
- Per step a fused 2-column capture matmul produces S_s = colsum(P_s) and
  D_s = stop-dot(P_s); ACT stages captures to SBUF chunks, DMA'd to DRAM.
- Host reconstructs  logZ_b = log D_{len_b} + cum(lse + chat)  and computes
  the gold-path score exactly; returns mean(logZ - gold) as f32.

The emission structure is shaped by a hardware constraint: this toolchain's
walrus accepts at most ONE sync-wait per ISA instruction. Joiner ops
(tiny TTs / ldweights) make each engine observe other engines' semaphores
so every compute instruction needs at most one wait; a post-build pass
splits the framework's multi-wait final Drain into single-wait clones.
"""
from contextlib import ExitStack
import copy
import time as _time
import numpy as np
import ml_dtypes

import concourse.bass as bass
import concourse.mybir as mybir
import concourse.tile as tile
from concourse.bass_utils import run_bass_kernel_spmd

BF16 = mybir.dt.bfloat16
F32 = mybir.dt.float32
ALU = mybir.AluOpType

B, T, K = 512, 512, 64
START, STOP = K - 2, K - 1
NCORES = 8
BC = B // NCORES

G = 2        # independent batch groups per core (chains interleave)
CAPN = 4     # steps per capture matmul
CHUNK = 16   # steps per F DMA chunk
WCHUNK = 64  # capture slots per Wc chunk


def _split_multi_waits(nc):
    """walrus accepts one sync-wait per instruction; split any multi-wait
    instruction (the framework's final Drain) into single-wait clones."""
    for fn in nc.m.functions:
        for blk in fn.blocks:
            out = []
            changed = False
            for inst in blk.instructions:
                si = inst.sync_info
                if si is not None and len(si.on_wait) > 1:
                    waits = list(si.on_wait)
                    for j, w in enumerate(waits[:-1]):
                        cl = copy.deepcopy(inst)
                        cl.name = f"{inst.name}_w{j}"
                        cl.sync_info = mybir.SyncInfo(on_wait=[w], on_update=[])
                        out.append(cl)
                        changed = True
                    si.on_wait = [waits[-1]]
                out.append(inst)
            if changed:
                blk.instructions = out


def _build_nc(T=T, G=G, CAPN=CAPN, CHUNK=CHUNK, WCHUNK=WCHUNK):
    assert T % CHUNK == 0 and T % WCHUNK == 0 and WCHUNK % CAPN == 0
    W = 64 // G
    NCH = T // CHUNK
    NWC = T // WCHUNK + 1
    nc = bass.Bass("TRN2", target_bir_lowering=False, debug=False)

    consts_d = nc.dram_tensor("consts", [64, 130], BF16, kind="ExternalInput").ap()
    fexp_d = nc.dram_tensor("fexp", [NCH, 64, CHUNK * 64], BF16, kind="ExternalInput").ap()
    wout_d = nc.dram_tensor("wout", [NWC, 2, WCHUNK * 64], BF16, kind="ExternalOutput").ap()

    with tile.TileContext(nc) as tc, ExitStack() as ctx:
        cpool = ctx.enter_context(tc.tile_pool(name="const", bufs=1))
        fcpool = ctx.enter_context(tc.tile_pool(name="fc", bufs=NCH))
        pppool = ctx.enter_context(tc.tile_pool(name="pp", bufs=8))
        wcpool = ctx.enter_context(tc.tile_pool(name="wc", bufs=NWC))
        jpool = ctx.enter_context(tc.tile_pool(name="join", bufs=2))
        vb = 3 if G == 1 else 2
        vpool = ctx.enter_context(tc.tile_pool(name="v", bufs=vb, space="PSUM"))
        capool = ctx.enter_context(tc.tile_pool(name="cap", bufs=1, space="PSUM"))

        ct = cpool.tile([64, 130], BF16)
        nc.sync.dma_start(ct[:, :], consts_d)
        ehat = ct[:, 0:66]

        # persistent capture psum banks: NCAPT tiles x 4 slots, striped by
        # flush index so same-t sibling flushes hit different banks
        CSL = CAPN * W
        NCAPT = 4 if G == 2 else 2
        cap_tiles = [capool.tile([2, 4 * CSL], F32, tag=f"capt{i}", name=f"capt{i}")
                     for i in range(NCAPT)]
        flush_ctr = [0]
        NTAG = NCAPT * 4 + 4
        wtpool = ctx.enter_context(tc.tile_pool(name="wt", bufs=NTAG))
        wtag_tiles = []
        # PE warmup: absorb the consts-DMA wait into PE's observed ticks
        nc.tensor.ldweights(ct[0:1, 0:1])

        fc_tiles = []
        for c in range(NCH):
            fc = fcpool.tile([64, CHUNK * 64], BF16, tag="fc", name=f"fc{c}")
            nc.sync.dma_start(fc[:, :], fexp_d[c])
            # DVE joiner: observe this chunk's DMA so U-mults need no DMA wait
            jt = jpool.tile([1, 2], BF16, tag="j", name=f"jt{c}", bufs=NCH)
            nc.vector.tensor_tensor(jt[:, :], fc[0:1, 0:2], fc[0:1, 0:2], ALU.mult)
            fc_tiles.append(fc)

        def f_slice(t, g):
            if t > T:
                t -= 4          # junk tail steps reuse old emission data
            c, tl = (t - 1) // CHUNK, (t - 1) % CHUNK
            return fc_tiles[c][:, tl * 64 + g * W: tl * 64 + (g + 1) * W]

        pp_cur = [None] * G
        cap_src = [dict() for _ in range(G)]
        wc_tiles = []

        def wc_for(chunk):
            while len(wc_tiles) <= chunk:
                wc_tiles.append(wcpool.tile([2, WCHUNK * 64], BF16, tag="wc",
                                            name=f"wc{len(wc_tiles)}"))
            return wc_tiles[chunk]

        for g in range(G):
            pp = pppool.tile([64, CAPN * W], BF16, tag=f"pp{g}", name=f"pp{g}_0")
            pp_cur[g] = pp
            nc.vector.tensor_tensor(pp[:, 0:W], ct[:, 66 + g * W: 66 + (g + 1) * W],
                                    ct[:, 66 + g * W: 66 + (g + 1) * W], ALU.max)
            cap_src[g][0] = (pp, 0)

        def cap_flush(g, s_hi):
            pp = pp_cur[g]
            s_lo = s_hi - (s_hi % CAPN)
            n = s_hi - s_lo + 1
            k = flush_ctr[0]; flush_ctr[0] += 1
            capt = cap_tiles[k % NCAPT]
            co = ((k // NCAPT) % 4) * CSL
            cap = capt[:, co:co + CSL]
            if k >= NCAPT:
                # observe the newest ACT copy touching this psum bank: a
                # no-output weight load waiting on its bf16 tag write
                nc.tensor.ldweights(wtag_tiles[k - NCAPT][0:1, 0:2])
            nc.tensor.matmul(cap[:, 0:n * W], lhsT=ehat[:, 64:66],
                             rhs=pp[:, 0:n * W], start=True, stop=True)
            wci = wc_for(s_lo // WCHUNK)
            view = wci[:, :].rearrange("p (s b) -> p s b", b=64)
            sl = s_lo % WCHUNK
            dst = view[:, sl:sl + n, g * W:(g + 1) * W]
            src = cap[:, 0:n * W].rearrange("p (s b) -> p s b", b=W)
            nc.scalar.copy(dst, src)
            wt = wtpool.tile([1, 2], BF16, tag="wt", name=f"wt{len(wtag_tiles)}")
            nc.scalar.copy(wt[:, :], cap[0:1, 0:2])
            wtag_tiles.append(wt)

        for t in range(1, T + 4):
            for g in range(G):
                pp_prev, slot_prev = cap_src[g][t - 1]
                v = vpool.tile([64, W], F32, tag=f"v{g}", name=f"v{g}_{t}")
                nc.tensor.matmul(
                    v[:, :], lhsT=ehat[:, 0:64],
                    rhs=pp_prev[:, slot_prev * W:(slot_prev + 1) * W],
                    start=True, stop=True)
                if t % CAPN == 0:
                    pp_cur[g] = pppool.tile([64, CAPN * W], BF16, tag=f"pp{g}",
                                            name=f"pp{g}_{t}")
                pp = pp_cur[g]
                slot = t % CAPN
                nc.vector.tensor_tensor(pp[:, slot * W:(slot + 1) * W],
                                        v[:, :], f_slice(t, g), ALU.mult)
                cap_src[g][t] = (pp, slot)
                if slot == CAPN - 1:
                    cap_flush(g, t)
            if t % WCHUNK == WCHUNK - 1:
                c = t // WCHUNK
                eng = nc.gpsimd if c % 2 == 0 else nc.scalar
                eng.dma_start(wout_d[c], wc_for(c)[:, :])
        c = T // WCHUNK
        nfin = 4                 # slots s=512..515 (junk beyond 512)
        nc.gpsimd.dma_start(wout_d[c][:, 0:nfin * 64], wc_for(c)[:, 0:nfin * 64])
    _split_multi_waits(nc)
    return nc


# ---------------- host pre/post processing ----------------

def _prep_core_inputs(feats_core, transitions):
    """feats_core: (BC, T, K) f32 -> (fexp bf16 chunks, shift (T, BC) f64).

    F_t = softmax(feats_t) * exp(-chat_t); shift = lse_t + chat_t is what the
    host adds back per step (exact, fp64).

    All heavy math stays in f32 (the emissions are rounded to bf16 for the
    device anyway; the shift only needs to equal, in fp64, the log of the f32
    factors actually applied)."""
    E = np.exp(transitions.astype(np.float32))
    w = (E.sum(axis=1) / 64.0).astype(np.float32)
    f = feats_core.astype(np.float32)
    m = f.max(axis=2, keepdims=True)
    e = np.exp(f - m)
    s = e.sum(axis=2, keepdims=True)
    lse = (np.log(s[:, :, 0].astype(np.float64)) + m[:, :, 0].astype(np.float64)).T
    soft = e / s                                          # (BC, T, K) f32
    chat = np.log(soft @ w)                               # (BC, T) f32, BLAS
    soft *= np.exp(-chat)[:, :, None]
    shift = lse + chat.T.astype(np.float64)               # (T, BC) f64
    NCH = T // CHUNK
    # one strided copy: (BC, T, K) -> (NCH, K, CHUNK, BC), bf16 at the end
    fexp = soft.reshape(BC, NCH, CHUNK, K).transpose(1, 3, 2, 0) \
               .reshape(NCH, K, CHUNK * BC)
    return np.ascontiguousarray(fexp).astype(ml_dtypes.bfloat16), shift


def _make_consts(transitions):
    E = np.exp(transitions.astype(np.float32))
    ehat = np.zeros((K, 66), np.float32)
    ehat[:, 0:K] = E.T          # lhsT[j, i] = E[i, j]
    ehat[:, 64] = 1.0           # column-sum capture row (S)
    ehat[:, 65] = E[STOP, :]    # stop-dot capture row (D)
    pinit = np.zeros((K, K), np.float32)
    pinit[START, :] = 1.0
    return np.concatenate([ehat, pinit], axis=1).astype(ml_dtypes.bfloat16)


def _postprocess(wout, shift, lengths_core):
    NWC = T // WCHUNK + 1
    wout = np.asarray(wout).astype(np.float32)
    flat = wout.reshape(NWC, 2, WCHUNK, BC)
    D = flat[:, 1].reshape(-1, BC)[:T + 1]                # stop-dots, (T+1, BC)
    shift_cum = np.concatenate([np.zeros((1, BC)), np.cumsum(shift, axis=0)], axis=0)
    alpha = np.log(np.maximum(D.astype(np.float64), 1e-300)) + shift_cum
    idx = lengths_core.astype(np.int64)
    return alpha[idx, np.arange(BC)]


def _gold_score(feats, transitions, tags, lengths):
    Bb, Tt, _ = feats.shape
    t_idx = np.arange(Tt + 1)
    tags = tags.astype(np.int64)
    lengths = lengths.astype(np.int64)
    pad_start = np.concatenate([np.full((Bb, 1), START, tags.dtype), tags], axis=1)
    pad_stop = np.concatenate([tags, np.full((Bb, 1), STOP, tags.dtype)], axis=1)
    pad_stop = np.where(t_idx[None, :] >= lengths[:, None], STOP, pad_stop)
    trans_mask = (t_idx[None, :] <= lengths[:, None]).astype(np.float64)
    trans_score = np.sum(transitions[pad_stop, pad_start].astype(np.float64) * trans_mask, axis=1)
    emit_mask = (np.arange(Tt)[None, :] < lengths[:, None]).astype(np.float64)
    emit = np.take_along_axis(feats, tags[:, :, None], axis=2)[:, :, 0].astype(np.float64)
    emit_score = np.sum(emit * emit_mask, axis=1)
    return trans_score + emit_score


_NC_CACHE = {}


def _get_nc():
    if "nc" not in _NC_CACHE:
        _NC_CACHE["nc"] = _build_nc()
    return _NC_CACHE["nc"]


def kernel(feats, transitions, tags, lengths, _trace=False, _return_extra=False):
    feats = np.asarray(feats)
    transitions = np.asarray(transitions)
    tags = np.asarray(tags)
    lengths = np.asarray(lengths)

    consts = _make_consts(transitions)
    in_maps = []
    shifts = []
    for c in range(NCORES):
        fexp, shift = _prep_core_inputs(feats[c * BC:(c + 1) * BC], transitions)
        shifts.append(shift)
        in_maps.append({"consts": consts, "fexp": fexp})

    _t0 = _time.time()
    res = run_bass_kernel_spmd(_get_nc(), in_maps, core_ids=list(range(NCORES)),
                               trace=_trace)
    _dev_s = _time.time() - _t0

    fwd = np.zeros((B,), np.float64)
    for c in range(NCORES):
        wout = np.asarray(res.results[c]["wout"])
        fwd[c * BC:(c + 1) * BC] = _postprocess(wout, shifts[c],
                                                lengths[c * BC:(c + 1) * BC])

    gold = _gold_score(feats, transitions, tags, lengths)
    loss = np.float32(np.mean(fwd - gold))
    out = np.array(loss, dtype=np.float32)
    if _return_extra:
        return out, {"fwd": fwd, "gold": gold, "exec_time_ns": res.exec_time_ns,
                     "device_call_s": _dev_s}
    return out



# revision 10
# speedup vs baseline: 1.6574x; 1.6574x over previous
"""CRF loss kernel for 8x Trainium2 NeuronCores (Bass/Tile). Self-contained.

nn_CRF: loss = mean_b( logZ_b - gold_b ) for a linear-chain CRF with
B=512 sequences, T=512 steps, K=64 tags (START=62, STOP=63).

Strategy:
- Data-parallel over batch: core c takes sequences [64c, 64c+64).
- Device computes the forward algorithm in the exp domain:
      P_t = (E @ P_{t-1}) * F_t,      E = exp(transitions),
  with F_t laid out (tag, seq) and pre-scaled on host:
      F_t = softmax_i(feats[:, t-1, :]) * exp(-chat_t)
  where chat_t = log(sum_i softmax_i * rowmean(E)) estimates the per-step
  log-growth. On the real data this keeps all P magnitudes within e^{+-8}
  over 512 steps, so no on-device renormalization is needed; the host adds
  the exactly-known scale factors back in fp64.
- Per step a matmul advances P; every CAPN steps a 1-column capture
  matmul computes the stop-dots D_s = E[STOP, :] . P_s of the last CAPN
  states from the grouped pp tile; ACT stages captures to SBUF chunks,
  DMA'd to DRAM. Emissions ship as fp8 (e4m3) and feed the DVE multiply
  directly; the loss tolerance (2e-2 rel on a ~1.4e3 loss) dwarfs the
  quantization noise. (Sharing one PSUM tile between several matmul
  writers and DVE readers makes the Tile framework emit multi-wait
  instructions walrus rejects, and the DVE cannot read single-partition
  PSUM slices at all — hence this capture structure, single-writer PSUM
  tiles, and the tag/ldweights joiners below.)
- Host reconstructs  logZ_b = log D_{len_b} + cum(lse + chat)  and computes
  the gold-path score exactly; returns mean(logZ - gold) as f32.

The end-to-end device call is dominated by axon dispatch + transfer, so the
kernel is shaped to minimize bytes shipped and BIR module size. A process-
wide persistent JAX compilation cache skips the NEFF recompile that
run_bass_kernel_spmd would otherwise redo on every invocation.

Hardware constraint: this toolchain's walrus accepts at most ONE sync-wait
per ISA instruction. Per-chunk DVE joiner ops make the vector engine
observe emission-DMA semaphores early so every compute instruction needs at
most one wait; a post-build pass splits the framework's multi-wait final
Drain into single-wait clones.
"""
from contextlib import ExitStack
import copy
import os
import tempfile
import time as _time
import numpy as np
import ml_dtypes

import jax
jax.config.update("jax_compilation_cache_dir",
                  os.path.join(tempfile.gettempdir(), "bass_jax_ccache"))
jax.config.update("jax_persistent_cache_min_compile_time_secs", 0.0)
jax.config.update("jax_persistent_cache_min_entry_size_bytes", 0)

import concourse.bass as bass
import concourse.mybir as mybir
import concourse.tile as tile
from concourse.bass_utils import run_bass_kernel_spmd

BF16 = mybir.dt.bfloat16
FP8 = mybir.dt.float8e4
F32 = mybir.dt.float32
ALU = mybir.AluOpType
NP_FP8 = mybir.dt.np(FP8)

B, T, K = 512, 512, 64
START, STOP = K - 2, K - 1
NCORES = 8
BC = B // NCORES

CHUNK = 32   # steps per emission DMA chunk (2KB/partition in fp8)
WCHUNK = 64  # D slots per output DMA chunk
CAPN = 4     # steps per capture matmul
NCH = T // CHUNK
NWC = T // WCHUNK + 1      # 513 slots: D_0 .. D_T


def _split_multi_waits(nc):
    """walrus accepts one sync-wait per instruction; split any multi-wait
    instruction (the framework's final Drain) into single-wait clones."""
    for fn in nc.m.functions:
        for blk in fn.blocks:
            out = []
            changed = False
            for inst in blk.instructions:
                si = inst.sync_info
                if si is not None and len(si.on_wait) > 1:
                    waits = list(si.on_wait)
                    for j, w in enumerate(waits[:-1]):
                        cl = copy.deepcopy(inst)
                        cl.name = f"{inst.name}_w{j}"
                        cl.sync_info = mybir.SyncInfo(on_wait=[w], on_update=[])
                        out.append(cl)
                        changed = True
                    si.on_wait = [waits[-1]]
                out.append(inst)
            if changed:
                blk.instructions = out
    return nc


def _build_nc():
    nc = bass.Bass("TRN2", target_bir_lowering=False, debug=False)

    consts_d = nc.dram_tensor("consts", [64, 129], BF16, kind="ExternalInput").ap()
    fexp_d = nc.dram_tensor("fexp", [NCH, 64, CHUNK * 64], FP8,
                            kind="ExternalInput").ap()
    wout_d = nc.dram_tensor("wout", [NWC, 1, WCHUNK * 64], BF16,
                            kind="ExternalOutput").ap()

    CSL = CAPN * 64              # f32 elements per capture slot
    NCAPT = 4                    # capture PSUM tiles (1 bank each, 2 slots)
    SLOTS = 2                    # capture slots per PSUM tile (2KB bank cap)

    with tile.TileContext(nc) as tc, ExitStack() as ctx:
        cpool = ctx.enter_context(tc.tile_pool(name="const", bufs=1))
        fcpool = ctx.enter_context(tc.tile_pool(name="fc", bufs=NCH))
        pppool = ctx.enter_context(tc.tile_pool(name="pp", bufs=8))
        wcpool = ctx.enter_context(tc.tile_pool(name="wc", bufs=NWC))
        jpool = ctx.enter_context(tc.tile_pool(name="join", bufs=2))
        vpool = ctx.enter_context(tc.tile_pool(name="v", bufs=3, space="PSUM"))
        capool = ctx.enter_context(tc.tile_pool(name="cap", bufs=1, space="PSUM"))

        ct = cpool.tile([64, 129], BF16)
        nc.sync.dma_start(ct[:, :], consts_d)
        ehat = ct[:, 0:65]           # E^T | E[STOP, :] column

        # persistent capture psum banks, striped by flush index so
        # consecutive flushes hit different banks
        cap_tiles = [capool.tile([1, SLOTS * CSL], F32, tag=f"capt{i}",
                                 name=f"capt{i}") for i in range(NCAPT)]
        flush_ctr = [0]
        wtpool = ctx.enter_context(tc.tile_pool(name="wt", bufs=NCAPT * SLOTS + 4))
        wtag_tiles = []
        # PE warmup: absorb the consts-DMA wait into PE's observed ticks
        nc.tensor.ldweights(ct[0:1, 0:1])

        fc_tiles = []
        for c in range(NCH):
            fc = fcpool.tile([64, CHUNK * 64], FP8, tag="fc", name=f"fc{c}")
            nc.sync.dma_start(fc[:, :], fexp_d[c])
            # DVE joiner: observe this chunk's DMA so step mults need no DMA wait
            jt = jpool.tile([1, 2], BF16, tag="j", name=f"jt{c}", bufs=NCH)
            nc.vector.tensor_tensor(jt[:, :], fc[0:1, 0:2], fc[0:1, 0:2], ALU.mult)
            fc_tiles.append(fc)

        def f_slice(t):
            if t > T:
                t -= CAPN       # junk tail steps reuse old emission data
            c, tl = (t - 1) // CHUNK, (t - 1) % CHUNK
            return fc_tiles[c][:, tl * 64:(tl + 1) * 64]

        wc_tiles = [wcpool.tile([1, WCHUNK * 64], BF16, tag="wc", name=f"wc{i}")
                    for i in range(NWC)]

        pp_cur = [None]
        cap_src = {}

        pp = pppool.tile([64, CAPN * 64], BF16, tag="pp", name="pp_0")
        pp_cur[0] = pp
        nc.vector.tensor_tensor(pp[:, 0:64], ct[:, 65:129], ct[:, 65:129], ALU.max)
        cap_src[0] = (pp, 0)

        def cap_flush(s_hi):
            pp = pp_cur[0]
            s_lo = s_hi - (s_hi % CAPN)
            n = s_hi - s_lo + 1
            k = flush_ctr[0]; flush_ctr[0] += 1
            capt = cap_tiles[k % NCAPT]
            co = ((k // NCAPT) % SLOTS) * CSL
            cap = capt[:, co:co + n * 64]
            if k >= NCAPT:
                # observe the newest ACT copy touching this psum bank: a
                # no-output weight load waiting on its bf16 tag write
                nc.tensor.ldweights(wtag_tiles[k - NCAPT][0:1, 0:2])
            nc.tensor.matmul(cap[:, :], lhsT=ehat[:, 64:65],
                             rhs=pp[:, 0:n * 64], start=True, stop=True)
            wci = wc_tiles[s_lo // WCHUNK]
            off = (s_lo % WCHUNK) * 64
            nc.scalar.copy(wci[:, off:off + n * 64], cap[:, :])
            wt = wtpool.tile([1, 2], BF16, tag="wt", name=f"wt{k}")
            nc.scalar.copy(wt[:, :], cap[0:1, 0:2])
            wtag_tiles.append(wt)
            if (s_lo + n) % WCHUNK == 0:
                c = s_lo // WCHUNK
                eng = nc.gpsimd if c % 2 == 0 else nc.scalar
                eng.dma_start(wout_d[c], wci[:, :])

        for t in range(1, T + CAPN):
            pp_prev, slot_prev = cap_src[t - 1]
            v = vpool.tile([64, 64], F32, tag="v", name=f"v_{t}")
            nc.tensor.matmul(
                v[:, :], lhsT=ehat[:, 0:64],
                rhs=pp_prev[:, slot_prev * 64:(slot_prev + 1) * 64],
                start=True, stop=True)
            if t % CAPN == 0:
                pp_cur[0] = pppool.tile([64, CAPN * 64], BF16, tag="pp",
                                        name=f"pp_{t}")
            pp = pp_cur[0]
            slot = t % CAPN
            nc.vector.tensor_tensor(pp[:, slot * 64:(slot + 1) * 64],
                                    v[:, :], f_slice(t), ALU.mult)
            cap_src[t] = (pp, slot)
            if slot == CAPN - 1:
                cap_flush(t)
        # the tail flush wrote s = 512..515 into chunk 8; ship those 4 slots
        nc.gpsimd.dma_start(wout_d[NWC - 1][:, 0:CAPN * 64],
                            wc_tiles[NWC - 1][:, 0:CAPN * 64])
    return _split_multi_waits(nc)


# ---------------- host pre/post processing ----------------

def _prep_core_inputs(feats_core, transitions):
    """feats_core: (BC, T, K) f32 -> (fexp fp8 chunks, shift (T, BC) f64).

    F_t = softmax(feats_t) * exp(-chat_t); shift = lse_t + chat_t is what the
    host adds back per step (exact, fp64). Quantization of F to fp8 is plain
    emission noise, far under the loss tolerance."""
    E = np.exp(transitions.astype(np.float32))
    w = (E.sum(axis=1) / 64.0).astype(np.float32)
    f = feats_core.astype(np.float32)
    m = f.max(axis=2, keepdims=True)
    e = np.exp(f - m)
    s = e.sum(axis=2, keepdims=True)
    lse = (np.log(s[:, :, 0].astype(np.float64)) + m[:, :, 0].astype(np.float64)).T
    soft = e / s                                          # (BC, T, K) f32
    chat = np.log(soft @ w)                               # (BC, T) f32, BLAS
    soft *= np.exp(-chat)[:, :, None]
    shift = lse + chat.T.astype(np.float64)               # (T, BC) f64
    # one strided copy: (BC, T, K) -> (NCH, K, CHUNK, BC), fp8 at the end
    fexp = soft.reshape(BC, NCH, CHUNK, K).transpose(1, 3, 2, 0) \
               .reshape(NCH, K, CHUNK * BC)
    return np.ascontiguousarray(fexp).astype(NP_FP8), shift


def _make_consts(transitions):
    E = np.exp(transitions.astype(np.float32))
    ehat = np.zeros((K, 65), np.float32)
    ehat[:, 0:K] = E.T          # lhsT[j, i] = E[i, j]
    ehat[:, 64] = E[STOP, :]    # stop-dot capture column (D)
    pinit = np.zeros((K, K), np.float32)
    pinit[START, :] = 1.0
    return np.concatenate([ehat, pinit], axis=1).astype(ml_dtypes.bfloat16)


def _postprocess(wout, shift, lengths_core):
    wout = np.asarray(wout).astype(np.float32)
    D = wout.reshape(-1, BC)[:T + 1]                      # stop-dots, (T+1, BC)
    shift_cum = np.concatenate([np.zeros((1, BC)), np.cumsum(shift, axis=0)], axis=0)
    alpha = np.log(np.maximum(D.astype(np.float64), 1e-300)) + shift_cum
    idx = lengths_core.astype(np.int64)
    return alpha[idx, np.arange(BC)]


def _gold_score(feats, transitions, tags, lengths):
    Bb, Tt, _ = feats.shape
    t_idx = np.arange(Tt + 1)
    tags = tags.astype(np.int64)
    lengths = lengths.astype(np.int64)
    pad_start = np.concatenate([np.full((Bb, 1), START, tags.dtype), tags], axis=1)
    pad_stop = np.concatenate([tags, np.full((Bb, 1), STOP, tags.dtype)], axis=1)
    pad_stop = np.where(t_idx[None, :] >= lengths[:, None], STOP, pad_stop)
    trans_mask = (t_idx[None, :] <= lengths[:, None]).astype(np.float64)
    trans_score = np.sum(transitions[pad_stop, pad_start].astype(np.float64) * trans_mask, axis=1)
    emit_mask = (np.arange(Tt)[None, :] < lengths[:, None]).astype(np.float64)
    emit = np.take_along_axis(feats, tags[:, :, None], axis=2)[:, :, 0].astype(np.float64)
    emit_score = np.sum(emit * emit_mask, axis=1)
    return trans_score + emit_score


_NC_CACHE = {}


def _get_nc():
    if "nc" not in _NC_CACHE:
        _NC_CACHE["nc"] = _build_nc()
    return _NC_CACHE["nc"]


def kernel(feats, transitions, tags, lengths, _trace=False, _return_extra=False):
    feats = np.asarray(feats)
    transitions = np.asarray(transitions)
    tags = np.asarray(tags)
    lengths = np.asarray(lengths)

    consts = _make_consts(transitions)
    in_maps = []
    shifts = []
    for c in range(NCORES):
        fexp, shift = _prep_core_inputs(feats[c * BC:(c + 1) * BC], transitions)
        shifts.append(shift)
        in_maps.append({"consts": consts, "fexp": fexp})

    _t0 = _time.time()
    res = run_bass_kernel_spmd(_get_nc(), in_maps, core_ids=list(range(NCORES)),
                               trace=_trace)
    _dev_s = _time.time() - _t0

    fwd = np.zeros((B,), np.float64)
    for c in range(NCORES):
        wout = np.asarray(res.results[c]["wout"])
        fwd[c * BC:(c + 1) * BC] = _postprocess(wout, shifts[c],
                                                lengths[c * BC:(c + 1) * BC])

    gold = _gold_score(feats, transitions, tags, lengths)
    loss = np.float32(np.mean(fwd - gold))
    out = np.array(loss, dtype=np.float32)
    if _return_extra:
        return out, {"fwd": fwd, "gold": gold, "exec_time_ns": res.exec_time_ns,
                     "device_call_s": _dev_s}
    return out


# revision 23
# speedup vs baseline: 1.8739x; 1.1306x over previous
"""CRF loss kernel for 8x Trainium2 NeuronCores (Bass/Tile). Self-contained.

nn_CRF: loss = mean_b( logZ_b - gold_b ) for a linear-chain CRF with
B=512 sequences, T=512 steps, K=64 tags (START=62, STOP=63).

Strategy:
- Data-parallel over batch: core c takes sequences [64c, 64c+64).
- Device computes the forward algorithm in the exp domain:
      P_t = (E @ P_{t-1}) * F_t,      E = exp(transitions),
  with F_t laid out (tag, seq) and pre-scaled on host:
      F_t = softmax_i(feats[:, t-1, :]) * exp(-chat_t)
  where chat_t = log(sum_i softmax_i * rowmean(E)) estimates the per-step
  log-growth. On the real data this keeps all P magnitudes within e^{+-8}
  over 512 steps, so no on-device renormalization is needed; the host adds
  the exactly-known scale factors back in fp64.
- Per step a matmul advances P; every CAPN steps a 1-column capture
  matmul computes the stop-dots D_s = E[STOP, :] . P_s of the last CAPN
  states from the grouped pp tile; ACT stages captures to SBUF chunks,
  DMA'd to DRAM. Emissions ship as fp8 (e4m3) and feed the DVE multiply
  directly; the loss tolerance (2e-2 rel on a ~1.4e3 loss) dwarfs the
  quantization noise. (Sharing one PSUM tile between several matmul
  writers and DVE readers makes the Tile framework emit multi-wait
  instructions walrus rejects, and the DVE cannot read single-partition
  PSUM slices at all — hence this capture structure, single-writer PSUM
  tiles, and the tag/ldweights joiners below.)
- Host reconstructs  logZ_b = log D_{len_b} + cum(lse + chat)  and computes
  the gold-path score exactly; returns mean(logZ - gold) as f32.

The end-to-end device call is dominated by axon dispatch + transfer, so the
kernel is shaped to minimize bytes shipped and BIR module size. A process-
wide persistent JAX compilation cache skips the NEFF recompile that
run_bass_kernel_spmd would otherwise redo on every invocation.

Hardware constraint: this toolchain's walrus accepts at most ONE sync-wait
per ISA instruction. Per-chunk DVE joiner ops make the vector engine
observe emission-DMA semaphores early so every compute instruction needs at
most one wait; a post-build pass splits the framework's multi-wait final
Drain into single-wait clones.
"""
from contextlib import ExitStack
import copy
import os
import tempfile
import time as _time
import numpy as np
import ml_dtypes

import jax
jax.config.update("jax_compilation_cache_dir",
                  os.path.join(tempfile.gettempdir(), "bass_jax_ccache"))
jax.config.update("jax_persistent_cache_min_compile_time_secs", 0.0)
jax.config.update("jax_persistent_cache_min_entry_size_bytes", 0)

import concourse.bass as bass
import concourse.mybir as mybir
import concourse.tile as tile
from concourse.bass_utils import run_bass_kernel_spmd

BF16 = mybir.dt.bfloat16
FP8 = mybir.dt.float8e4
F32 = mybir.dt.float32
ALU = mybir.AluOpType
NP_FP8 = mybir.dt.np(FP8)

B, T, K = 512, 512, 64
START, STOP = K - 2, K - 1
NCORES = 8
BC = B // NCORES

CHUNK = 32   # steps per emission DMA chunk (2KB/partition in fp8)
WCHUNK = 64  # D slots per staging chunk
CAPN = 8     # steps per capture matmul
NCH = T // CHUNK
NWC = T // WCHUNK + 1      # 513 live slots: D_0 .. D_T (+7 junk tail)


def _split_multi_waits(nc):
    """walrus accepts one sync-wait per instruction; split any multi-wait
    instruction (the framework's final Drain) into single-wait clones."""
    for fn in nc.m.functions:
        for blk in fn.blocks:
            out = []
            changed = False
            for inst in blk.instructions:
                si = inst.sync_info
                if si is not None and len(si.on_wait) > 1:
                    waits = list(si.on_wait)
                    for j, w in enumerate(waits[:-1]):
                        cl = copy.deepcopy(inst)
                        cl.name = f"{inst.name}_w{j}"
                        cl.sync_info = mybir.SyncInfo(on_wait=[w], on_update=[])
                        out.append(cl)
                        changed = True
                    si.on_wait = [waits[-1]]
                out.append(inst)
            if changed:
                blk.instructions = out
    return nc


def _build_nc():
    nc = bass.Bass("TRN2", target_bir_lowering=False, debug=False)

    consts_d = nc.dram_tensor("consts", [64, 129], BF16, kind="ExternalInput").ap()
    fexp_d = nc.dram_tensor("fexp", [NCH, 64, CHUNK * 64], FP8,
                            kind="ExternalInput").ap()
    mask_d = nc.dram_tensor("mask", [1, NWC * WCHUNK * 64], FP8,
                            kind="ExternalInput").ap()
    wout_d = nc.dram_tensor("wout", [1, WCHUNK * 64], BF16,
                            kind="ExternalOutput").ap()

    CSL = CAPN * 64              # f32 elements per capture slot (one full bank)
    NCAPT = 4                    # capture PSUM tiles (1 bank each, 1 slot)
    SLOTS = 1

    with tile.TileContext(nc) as tc, ExitStack() as ctx:
        cpool = ctx.enter_context(tc.tile_pool(name="const", bufs=1))
        fcpool = ctx.enter_context(tc.tile_pool(name="fc", bufs=NCH))
        pppool = ctx.enter_context(tc.tile_pool(name="pp", bufs=8))
        wcpool = ctx.enter_context(tc.tile_pool(name="wc", bufs=NWC))
        jpool = ctx.enter_context(tc.tile_pool(name="join", bufs=2))
        vpool = ctx.enter_context(tc.tile_pool(name="v", bufs=3, space="PSUM"))
        capool = ctx.enter_context(tc.tile_pool(name="cap", bufs=1, space="PSUM"))

        ct = cpool.tile([64, 129], BF16)
        nc.sync.dma_start(ct[:, :], consts_d)
        ehat = ct[:, 0:65]           # E^T | E[STOP, :] column
        mt = cpool.tile([1, NWC * WCHUNK * 64], FP8, name="mask")
        nc.sync.dma_start(mt[:, :], mask_d)

        # persistent capture psum banks, striped by flush index so
        # consecutive flushes hit different banks
        cap_tiles = [capool.tile([1, SLOTS * CSL], F32, tag=f"capt{i}",
                                 name=f"capt{i}") for i in range(NCAPT)]
        flush_ctr = [0]
        wtpool = ctx.enter_context(tc.tile_pool(name="wt", bufs=NCAPT * SLOTS + 4))
        wtag_tiles = []
        # PE warmup: absorb the consts-DMA wait into PE's observed ticks
        nc.tensor.ldweights(ct[0:1, 0:1])

        fc_tiles = []
        for c in range(NCH):
            fc = fcpool.tile([64, CHUNK * 64], FP8, tag="fc", name=f"fc{c}")
            nc.sync.dma_start(fc[:, :], fexp_d[c])
            # DVE joiner: observe this chunk's DMA so step mults need no DMA wait
            jt = jpool.tile([1, 2], BF16, tag="j", name=f"jt{c}", bufs=NCH)
            nc.vector.tensor_tensor(jt[:, :], fc[0:1, 0:2], fc[0:1, 0:2], ALU.mult)
            fc_tiles.append(fc)
        # DVE joiner for the mask DMA
        jm = jpool.tile([1, 2], BF16, name="jm", bufs=NCH)
        nc.vector.tensor_tensor(jm[:, :], mt[0:1, 0:2], mt[0:1, 0:2], ALU.mult)

        def f_slice(t):
            if t > T:
                t -= CAPN       # junk tail steps reuse old emission data
            c, tl = (t - 1) // CHUNK, (t - 1) % CHUNK
            return fc_tiles[c][:, tl * 64:(tl + 1) * 64]

        wc_tiles = [wcpool.tile([1, WCHUNK * 64], BF16, tag="wc", name=f"wc{i}")
                    for i in range(NWC)]
        ppool = ctx.enter_context(tc.tile_pool(name="p", bufs=4))
        acc_state = [None]       # (acc tile, valid width)

        pp_cur = [None]
        cap_src = {}

        pp = pppool.tile([64, CAPN * 64], BF16, tag="pp", name="pp_0")
        pp_cur[0] = pp
        nc.vector.tensor_tensor(pp[:, 0:64], ct[:, 65:129], ct[:, 65:129], ALU.max)
        cap_src[0] = (pp, 0)

        def cap_flush(s_hi):
            pp = pp_cur[0]
            s_lo = s_hi - (s_hi % CAPN)
            n = s_hi - s_lo + 1
            k = flush_ctr[0]; flush_ctr[0] += 1
            capt = cap_tiles[k % NCAPT]
            cap = capt[:, 0:n * 64]
            if k >= NCAPT:
                # observe the newest ACT copy touching this psum bank: a
                # no-output weight load waiting on its bf16 tag write
                nc.tensor.ldweights(wtag_tiles[k - NCAPT][0:1, 0:2])
            nc.tensor.matmul(cap[:, :], lhsT=ehat[:, 64:65],
                             rhs=pp[:, 0:n * 64], start=True, stop=True)
            ci = s_lo // WCHUNK
            wci = wc_tiles[ci]
            off = (s_lo % WCHUNK) * 64
            nc.scalar.copy(wci[:, off:off + n * 64], cap[:, :])
            wt = wtpool.tile([1, 2], BF16, tag="wt", name=f"wt{k}")
            nc.scalar.copy(wt[:, :], cap[0:1, 0:2])
            wtag_tiles.append(wt)
            if (s_lo + n) % WCHUNK == 0 or s_hi == T + CAPN - 1:
                # chunk complete: select this chunk's contribution via the
                # one-hot length mask (exact: 0/1 in fp8, one hot slot per
                # seq) and fold it into the running accumulator
                w = off + n * 64 if s_hi == T + CAPN - 1 else WCHUNK * 64
                pt = ppool.tile([1, WCHUNK * 64], BF16, tag="p", name=f"p{ci}")
                nc.vector.tensor_tensor(
                    pt[:, 0:w], wci[:, 0:w],
                    mt[:, ci * WCHUNK * 64:ci * WCHUNK * 64 + w], ALU.mult)
                if acc_state[0] is None:
                    acc_state[0] = (pt, WCHUNK * 64)
                else:
                    acc, aw = acc_state[0]
                    na = ppool.tile([1, WCHUNK * 64], BF16, tag="p",
                                    name=f"acc{ci}")
                    nc.vector.tensor_tensor(na[:, 0:w], acc[:, 0:w],
                                            pt[:, 0:w], ALU.add)
                    if w < aw:
                        nc.vector.tensor_tensor(na[:, w:aw], acc[:, w:aw],
                                                acc[:, w:aw], ALU.max)
                    acc_state[0] = (na, aw)

        for t in range(1, T + CAPN):
            pp_prev, slot_prev = cap_src[t - 1]
            v = vpool.tile([64, 64], F32, tag="v", name=f"v_{t}")
            nc.tensor.matmul(
                v[:, :], lhsT=ehat[:, 0:64],
                rhs=pp_prev[:, slot_prev * 64:(slot_prev + 1) * 64],
                start=True, stop=True)
            if t % CAPN == 0:
                pp_cur[0] = pppool.tile([64, CAPN * 64], BF16, tag="pp",
                                        name=f"pp_{t}")
            pp = pp_cur[0]
            slot = t % CAPN
            nc.vector.tensor_tensor(pp[:, slot * 64:(slot + 1) * 64],
                                    v[:, :], f_slice(t), ALU.mult)
            cap_src[t] = (pp, slot)
            if slot == CAPN - 1:
                cap_flush(t)
        # ship the [WCHUNK, BC] accumulator: exactly one nonzero per column,
        # the selected stop-dot D_{len_b}
        acc, aw = acc_state[0]
        nc.gpsimd.dma_start(wout_d, acc[:, :])
    return _split_multi_waits(nc)


# ---------------- host pre/post processing ----------------

def _prep_core_inputs(feats_core, transitions):
    """feats_core: (BC, T, K) f32 -> (fexp fp8 chunks, shift (T, BC) f64).

    F_t = softmax(feats_t) * exp(-chat_t); shift = lse_t + chat_t is what the
    host adds back per step (exact, fp64). Quantization of F to fp8 is plain
    emission noise, far under the loss tolerance."""
    E = np.exp(transitions.astype(np.float32))
    w = (E.sum(axis=1) / 64.0).astype(np.float32)
    f = feats_core.astype(np.float32)
    m = f.max(axis=2, keepdims=True)
    e = np.exp(f - m)
    s = e.sum(axis=2, keepdims=True)
    lse = (np.log(s[:, :, 0].astype(np.float64)) + m[:, :, 0].astype(np.float64)).T
    soft = e / s                                          # (BC, T, K) f32
    chat = np.log(soft @ w)                               # (BC, T) f32, BLAS
    soft *= np.exp(-chat)[:, :, None]
    shift = lse + chat.T.astype(np.float64)               # (T, BC) f64
    # one strided copy: (BC, T, K) -> (NCH, K, CHUNK, BC), fp8 at the end
    fexp = soft.reshape(BC, NCH, CHUNK, K).transpose(1, 3, 2, 0) \
               .reshape(NCH, K, CHUNK * BC)
    return np.ascontiguousarray(fexp).astype(NP_FP8), shift


def _make_consts(transitions):
    E = np.exp(transitions.astype(np.float32))
    ehat = np.zeros((K, 65), np.float32)
    ehat[:, 0:K] = E.T          # lhsT[j, i] = E[i, j]
    ehat[:, 64] = E[STOP, :]    # stop-dot capture column (D)
    pinit = np.zeros((K, K), np.float32)
    pinit[START, :] = 1.0
    return np.concatenate([ehat, pinit], axis=1).astype(ml_dtypes.bfloat16)


def _make_mask(lengths_core):
    """One-hot selection mask over D slots: hot at s = len_b for column b."""
    m = np.zeros((NWC * WCHUNK, BC), np.float32)
    m[lengths_core.astype(np.int64), np.arange(BC)] = 1.0
    return m.reshape(1, NWC * WCHUNK * 64).astype(NP_FP8)


def _postprocess(wout, shift, lengths_core):
    # wout: [1, WCHUNK*BC] — per (s % WCHUNK, b) partial sums, exactly one
    # nonzero per column b (the selected stop-dot D_{len_b})
    A = np.asarray(wout).astype(np.float64).reshape(WCHUNK, BC)
    D_sel = A.sum(axis=0)
    shift_cum = np.concatenate([np.zeros((1, BC)), np.cumsum(shift, axis=0)], axis=0)
    idx = lengths_core.astype(np.int64)
    return np.log(np.maximum(D_sel, 1e-300)) + shift_cum[idx, np.arange(BC)]


def _gold_score(feats, transitions, tags, lengths):
    Bb, Tt, _ = feats.shape
    t_idx = np.arange(Tt + 1)
    tags = tags.astype(np.int64)
    lengths = lengths.astype(np.int64)
    pad_start = np.concatenate([np.full((Bb, 1), START, tags.dtype), tags], axis=1)
    pad_stop = np.concatenate([tags, np.full((Bb, 1), STOP, tags.dtype)], axis=1)
    pad_stop = np.where(t_idx[None, :] >= lengths[:, None], STOP, pad_stop)
    trans_mask = (t_idx[None, :] <= lengths[:, None]).astype(np.float64)
    trans_score = np.sum(transitions[pad_stop, pad_start].astype(np.float64) * trans_mask, axis=1)
    emit_mask = (np.arange(Tt)[None, :] < lengths[:, None]).astype(np.float64)
    emit = np.take_along_axis(feats, tags[:, :, None], axis=2)[:, :, 0].astype(np.float64)
    emit_score = np.sum(emit * emit_mask, axis=1)
    return trans_score + emit_score


_NC_CACHE = {}


def _get_nc():
    if "nc" not in _NC_CACHE:
        _NC_CACHE["nc"] = _build_nc()
    return _NC_CACHE["nc"]


def kernel(feats, transitions, tags, lengths, _trace=False, _return_extra=False):
    feats = np.asarray(feats)
    transitions = np.asarray(transitions)
    tags = np.asarray(tags)
    lengths = np.asarray(lengths)

    consts = _make_consts(transitions)
    in_maps = []
    shifts = []
    for c in range(NCORES):
        fexp, shift = _prep_core_inputs(feats[c * BC:(c + 1) * BC], transitions)
        shifts.append(shift)
        mask = _make_mask(lengths[c * BC:(c + 1) * BC])
        in_maps.append({"consts": consts, "fexp": fexp, "mask": mask})

    _t0 = _time.time()
    res = run_bass_kernel_spmd(_get_nc(), in_maps, core_ids=list(range(NCORES)),
                               trace=_trace)
    _dev_s = _time.time() - _t0

    fwd = np.zeros((B,), np.float64)
    for c in range(NCORES):
        wout = np.asarray(res.results[c]["wout"])
        fwd[c * BC:(c + 1) * BC] = _postprocess(wout, shifts[c],
                                                lengths[c * BC:(c + 1) * BC])

    gold = _gold_score(feats, transitions, tags, lengths)
    loss = np.float32(np.mean(fwd - gold))
    out = np.array(loss, dtype=np.float32)
    if _return_extra:
        return out, {"fwd": fwd, "gold": gold, "exec_time_ns": res.exec_time_ns,
                     "device_call_s": _dev_s}
    return out


# revision 33
# speedup vs baseline: 2.8344x; 1.5126x over previous
"""CRF loss kernel for 8x Trainium2 NeuronCores (Bass/Tile). Self-contained.

nn_CRF: loss = mean_b( logZ_b - gold_b ) for a linear-chain CRF with
B=512 sequences, T=512 steps, K=64 tags (START=62, STOP=63).

Strategy:
- Data-parallel over batch: core c takes sequences [64c, 64c+64).
- Device computes the forward algorithm in the exp domain:
      P_t = (E @ P_{t-1}) * F_t,      E = exp(transitions),
  with F_t laid out (tag, seq) and pre-scaled on host:
      F_t = softmax_i(feats[:, t-1, :]) * exp(-chat_t)
  where chat_t = log(sum_i softmax_i * rowmean(E)) estimates the per-step
  log-growth. On the real data this keeps all P magnitudes within e^{+-8}
  over 512 steps, so no on-device renormalization is needed; the host adds
  the exactly-known scale factors back in fp64.
- Per step a matmul advances P; every CAPN steps a 1-column capture
  matmul computes the stop-dots D_s = E[STOP, :] . P_s of the last CAPN
  states from the grouped pp tile; ACT stages captures to SBUF chunks.
  A one-hot length mask (shipped fp8) selects D_{len_b} on device, so the
  output is a single [WCHUNK, BC] partial-sum tile per core.
- Emissions ship as packed 4-bit log codes (two per uint8): q ~=
  round(log1p(F/c)/s) with per-core scalars (s, ln c, c) shipped f32.
  The device unpacks nibbles (DVE and/shift), converts to bf16 (ACT),
  decodes F = c*exp(s*q) - c (ACT Exp with AP scale/bias, then a fused
  DVE subtract+relu), and feeds the fp8 result to the step multiply.
  Per-sequence quantization error reaches a few nats but averages out
  across B=512; measured loss error ~5e-5 vs the 2e-2 tolerance.
  (Sharing one PSUM tile between several matmul writers and DVE readers
  makes the Tile framework emit multi-wait instructions walrus rejects,
  and the DVE cannot read single-partition PSUM slices at all — hence
  this capture structure, single-writer PSUM tiles, and the tag/ldweights
  joiners below.)
- Host reconstructs  logZ_b = log D_{len_b} + cum(lse + chat)  and computes
  the gold-path score exactly; returns mean(logZ - gold) as f32.

The end-to-end device call is dominated by axon dispatch + transfer, so the
kernel is shaped to minimize bytes shipped and BIR module size. A process-
wide persistent JAX compilation cache skips the NEFF recompile that
run_bass_kernel_spmd would otherwise redo on every invocation.

Hardware constraint: this toolchain's walrus accepts at most ONE sync-wait
per ISA instruction. Per-chunk DVE joiner ops make the vector engine
observe emission-DMA semaphores early so every compute instruction needs at
most one wait; a post-build pass splits the framework's multi-wait final
Drain into single-wait clones.
"""
from contextlib import ExitStack
import copy
import os
import tempfile
import time as _time
import numpy as np
import ml_dtypes

import jax
jax.config.update("jax_compilation_cache_dir",
                  os.path.join(tempfile.gettempdir(), "bass_jax_ccache"))
jax.config.update("jax_persistent_cache_min_compile_time_secs", 0.0)
jax.config.update("jax_persistent_cache_min_entry_size_bytes", 0)

import concourse.bass as bass
import concourse.mybir as mybir
import concourse.tile as tile
from concourse.bass_utils import run_bass_kernel_spmd

BF16 = mybir.dt.bfloat16
FP8 = mybir.dt.float8e4
F32 = mybir.dt.float32
U8 = mybir.dt.uint8
ALU = mybir.AluOpType
AF = mybir.ActivationFunctionType
NP_FP8 = mybir.dt.np(FP8)

SQ = 0.45    # 4-bit log-quantization step (nats per code level)

B, T, K = 512, 512, 64
START, STOP = K - 2, K - 1
NCORES = 8
BC = B // NCORES

CHUNK = 32   # steps per emission DMA chunk (2KB/partition in fp8)
WCHUNK = 64  # D slots per staging chunk
CAPN = 8     # steps per capture matmul
NCH = T // CHUNK
NWC = T // WCHUNK + 1      # 513 live slots: D_0 .. D_T (+7 junk tail)


def _split_multi_waits(nc):
    """walrus accepts one sync-wait per instruction; split any multi-wait
    instruction (the framework's final Drain) into single-wait clones."""
    for fn in nc.m.functions:
        for blk in fn.blocks:
            out = []
            changed = False
            for inst in blk.instructions:
                si = inst.sync_info
                if si is not None and len(si.on_wait) > 1:
                    waits = list(si.on_wait)
                    for j, w in enumerate(waits[:-1]):
                        cl = copy.deepcopy(inst)
                        cl.name = f"{inst.name}_w{j}"
                        cl.sync_info = mybir.SyncInfo(on_wait=[w], on_update=[])
                        out.append(cl)
                        changed = True
                    si.on_wait = [waits[-1]]
                out.append(inst)
            if changed:
                blk.instructions = out
    return nc


def _build_nc():
    nc = bass.Bass("TRN2", target_bir_lowering=False, debug=False)

    consts_d = nc.dram_tensor("consts", [64, 129], BF16, kind="ExternalInput").ap()
    qconsts_d = nc.dram_tensor("qconsts", [64, 3], F32, kind="ExternalInput").ap()
    fexp_d = nc.dram_tensor("fexp", [NCH, 64, (CHUNK // 2) * 64], U8,
                            kind="ExternalInput").ap()
    mask_d = nc.dram_tensor("mask", [1, NWC * WCHUNK * 64], FP8,
                            kind="ExternalInput").ap()
    wout_d = nc.dram_tensor("wout", [1, WCHUNK * 64], BF16,
                            kind="ExternalOutput").ap()

    CSL = CAPN * 64              # f32 elements per capture slot (one full bank)
    NCAPT = 4                    # capture PSUM tiles (1 bank each, 1 slot)
    SLOTS = 1

    HCH = (CHUNK // 2) * 64      # packed bytes per partition per chunk

    with tile.TileContext(nc) as tc, ExitStack() as ctx:
        cpool = ctx.enter_context(tc.tile_pool(name="const", bufs=1))
        upool = ctx.enter_context(tc.tile_pool(name="u", bufs=NCH))
        nbpool = ctx.enter_context(tc.tile_pool(name="nb", bufs=4))
        qbpool = ctx.enter_context(tc.tile_pool(name="qb", bufs=4))
        qepool = ctx.enter_context(tc.tile_pool(name="qe", bufs=4))
        fcpool = ctx.enter_context(tc.tile_pool(name="fc", bufs=6))
        pppool = ctx.enter_context(tc.tile_pool(name="pp", bufs=8))
        wcpool = ctx.enter_context(tc.tile_pool(name="wc", bufs=NWC))
        jpool = ctx.enter_context(tc.tile_pool(name="join", bufs=2))
        vpool = ctx.enter_context(tc.tile_pool(name="v", bufs=3, space="PSUM"))
        capool = ctx.enter_context(tc.tile_pool(name="cap", bufs=1, space="PSUM"))

        ct = cpool.tile([64, 129], BF16)
        nc.sync.dma_start(ct[:, :], consts_d)
        ehat = ct[:, 0:65]           # E^T | E[STOP, :] column
        qc = cpool.tile([64, 3], F32, name="qc")
        nc.sync.dma_start(qc[:, :], qconsts_d)
        mt = cpool.tile([1, NWC * WCHUNK * 64], FP8, name="mask")
        nc.sync.dma_start(mt[:, :], mask_d)
        # joiners: DVE and ACT observe the qconsts DMA once, so the decode
        # ops below carry no DMA waits
        jq = cpool.tile([1, 2], BF16, name="jq")
        nc.vector.tensor_tensor(jq[:, :], qc[0:1, 0:2], qc[0:1, 0:2], ALU.mult)
        ja = cpool.tile([1, 2], BF16, name="ja")
        nc.scalar.copy(ja[:, :], qc[0:1, 0:2])

        # persistent capture psum banks, striped by flush index so
        # consecutive flushes hit different banks
        cap_tiles = [capool.tile([1, SLOTS * CSL], F32, tag=f"capt{i}",
                                 name=f"capt{i}") for i in range(NCAPT)]
        flush_ctr = [0]
        wtpool = ctx.enter_context(tc.tile_pool(name="wt", bufs=NCAPT * SLOTS + 4))
        wtag_tiles = []
        # PE warmup: absorb the consts-DMA wait into PE's observed ticks
        nc.tensor.ldweights(ct[0:1, 0:1])

        # all packed chunks DMA'd up front (distinct tiles: no DMA waits)
        u_tiles = []
        for c in range(NCH):
            u = upool.tile([64, HCH], U8, tag="u", name=f"u{c}")
            nc.sync.dma_start(u[:, :], fexp_d[c])
            u_tiles.append(u)

        # 4-bit decode, one chunk at a time (interleaved with the step loop
        # so rotating-pool reuse is separated by >=32 steps of engine work):
        # unpack nibbles, convert to bf16, map codes through
        # F = c*exp(s*q) - c (clamped at 0), stage as fp8
        fe_tiles, fo_tiles = {}, {}

        def decode_chunk(c):
            u = u_tiles[c]
            lo8 = nbpool.tile([64, HCH], U8, tag="nb", name=f"lo8_{c}")
            hi8 = nbpool.tile([64, HCH], U8, tag="nb", name=f"hi8_{c}")
            nc.vector.tensor_scalar(lo8[:, :], u[:, :], 15, None, ALU.bitwise_and)
            nc.vector.tensor_scalar(hi8[:, :], u[:, :], 4, None,
                                    ALU.logical_shift_right)
            lob = qbpool.tile([64, HCH], BF16, tag="qb", name=f"lob{c}")
            hib = qbpool.tile([64, HCH], BF16, tag="qb", name=f"hib{c}")
            nc.scalar.copy(lob[:, :], lo8[:, :])
            nc.scalar.copy(hib[:, :], hi8[:, :])
            fe = qepool.tile([64, HCH], FP8, tag="qe", name=f"fe{c}")
            fo = qepool.tile([64, HCH], FP8, tag="qe", name=f"fo{c}")
            nc.scalar.activation(fe[:, :], lob[:, :], AF.Exp,
                                 bias=qc[:, 1:2], scale=qc[:, 0:1])
            nc.scalar.activation(fo[:, :], hib[:, :], AF.Exp,
                                 bias=qc[:, 1:2], scale=qc[:, 0:1])
            fes = fcpool.tile([64, HCH], FP8, tag="fc", name=f"fes{c}")
            fos = fcpool.tile([64, HCH], FP8, tag="fc", name=f"fos{c}")
            nc.vector.tensor_scalar(fes[:, :], fe[:, :], qc[:, 2:3], 0.0,
                                    ALU.subtract, ALU.max)
            nc.vector.tensor_scalar(fos[:, :], fo[:, :], qc[:, 2:3], 0.0,
                                    ALU.subtract, ALU.max)
            fe_tiles[c] = fes
            fo_tiles[c] = fos

        decode_chunk(0)
        decode_chunk(1)
        # DVE joiner for the mask DMA
        jm = jpool.tile([1, 2], BF16, name="jm", bufs=2)
        nc.vector.tensor_tensor(jm[:, :], mt[0:1, 0:2], mt[0:1, 0:2], ALU.mult)

        def f_slice(t):
            if t > T:
                t -= CAPN       # junk tail steps reuse old emission data
            c, tl = (t - 1) // CHUNK, (t - 1) % CHUNK
            tiles = fe_tiles if tl % 2 == 0 else fo_tiles
            j = tl // 2
            return tiles[c][:, j * 64:(j + 1) * 64]

        wc_tiles = [wcpool.tile([1, WCHUNK * 64], BF16, tag="wc", name=f"wc{i}")
                    for i in range(NWC)]
        ppool = ctx.enter_context(tc.tile_pool(name="p", bufs=4))
        acc_state = [None]       # (acc tile, valid width)

        pp_cur = [None]
        cap_src = {}

        pp = pppool.tile([64, CAPN * 64], BF16, tag="pp", name="pp_0")
        pp_cur[0] = pp
        nc.vector.tensor_tensor(pp[:, 0:64], ct[:, 65:129], ct[:, 65:129], ALU.max)
        cap_src[0] = (pp, 0)

        def cap_flush(s_hi):
            pp = pp_cur[0]
            s_lo = s_hi - (s_hi % CAPN)
            n = s_hi - s_lo + 1
            k = flush_ctr[0]; flush_ctr[0] += 1
            capt = cap_tiles[k % NCAPT]
            cap = capt[:, 0:n * 64]
            if k >= NCAPT:
                # observe the newest ACT copy touching this psum bank: a
                # no-output weight load waiting on its bf16 tag write
                nc.tensor.ldweights(wtag_tiles[k - NCAPT][0:1, 0:2])
            nc.tensor.matmul(cap[:, :], lhsT=ehat[:, 64:65],
                             rhs=pp[:, 0:n * 64], start=True, stop=True)
            ci = s_lo // WCHUNK
            wci = wc_tiles[ci]
            off = (s_lo % WCHUNK) * 64
            nc.scalar.copy(wci[:, off:off + n * 64], cap[:, :])
            wt = wtpool.tile([1, 2], BF16, tag="wt", name=f"wt{k}")
            nc.scalar.copy(wt[:, :], cap[0:1, 0:2])
            wtag_tiles.append(wt)
            if (s_lo + n) % WCHUNK == 0 or s_hi == T + CAPN - 1:
                # chunk complete: select this chunk's contribution via the
                # one-hot length mask (exact: 0/1 in fp8, one hot slot per
                # seq) and fold it into the running accumulator
                w = off + n * 64 if s_hi == T + CAPN - 1 else WCHUNK * 64
                pt = ppool.tile([1, WCHUNK * 64], BF16, tag="p", name=f"p{ci}")
                nc.vector.tensor_tensor(
                    pt[:, 0:w], wci[:, 0:w],
                    mt[:, ci * WCHUNK * 64:ci * WCHUNK * 64 + w], ALU.mult)
                if acc_state[0] is None:
                    acc_state[0] = (pt, WCHUNK * 64)
                else:
                    acc, aw = acc_state[0]
                    na = ppool.tile([1, WCHUNK * 64], BF16, tag="p",
                                    name=f"acc{ci}")
                    nc.vector.tensor_tensor(na[:, 0:w], acc[:, 0:w],
                                            pt[:, 0:w], ALU.add)
                    if w < aw:
                        nc.vector.tensor_tensor(na[:, w:aw], acc[:, w:aw],
                                                acc[:, w:aw], ALU.max)
                    acc_state[0] = (na, aw)

        for t in range(1, T + CAPN):
            if t > 1 and t % CHUNK == 1:
                cnext = (t - 1) // CHUNK + 1
                if cnext < NCH:
                    decode_chunk(cnext)
            pp_prev, slot_prev = cap_src[t - 1]
            v = vpool.tile([64, 64], F32, tag="v", name=f"v_{t}")
            nc.tensor.matmul(
                v[:, :], lhsT=ehat[:, 0:64],
                rhs=pp_prev[:, slot_prev * 64:(slot_prev + 1) * 64],
                start=True, stop=True)
            if t % CAPN == 0:
                pp_cur[0] = pppool.tile([64, CAPN * 64], BF16, tag="pp",
                                        name=f"pp_{t}")
            pp = pp_cur[0]
            slot = t % CAPN
            nc.vector.tensor_tensor(pp[:, slot * 64:(slot + 1) * 64],
                                    v[:, :], f_slice(t), ALU.mult)
            cap_src[t] = (pp, slot)
            if slot == CAPN - 1:
                cap_flush(t)
        # ship the [WCHUNK, BC] accumulator: exactly one nonzero per column,
        # the selected stop-dot D_{len_b}
        acc, aw = acc_state[0]
        nc.gpsimd.dma_start(wout_d, acc[:, :])
    return _split_multi_waits(nc)


# ---------------- host pre/post processing ----------------

def _prep_core_inputs(feats_core, transitions):
    """feats_core: (BC, T, K) f32 -> (packed 4-bit codes, qconsts, shift).

    F_t = softmax(feats_t) * exp(-chat_t); shift = lse_t + chat_t is what the
    host adds back per step (exact, fp64). F is log-quantized to 4-bit codes
    q = round(log1p(F/c)/SQ); the device decodes c*exp(SQ*q) - c. The
    quantization is plain emission noise, far under the loss tolerance."""
    E = np.exp(transitions.astype(np.float32))
    w = (E.sum(axis=1) / 64.0).astype(np.float32)
    f = feats_core.astype(np.float32)
    m = f.max(axis=2, keepdims=True)
    e = np.exp(f - m)
    s = e.sum(axis=2, keepdims=True)
    lse = (np.log(s[:, :, 0].astype(np.float64)) + m[:, :, 0].astype(np.float64)).T
    soft = e / s                                          # (BC, T, K) f32
    chat = np.log(soft @ w)                               # (BC, T) f32, BLAS
    soft *= np.exp(-chat)[:, :, None]
    shift = lse + chat.T.astype(np.float64)               # (T, BC) f64
    c = float(soft.max()) / float(np.expm1(SQ * 15))
    q = np.clip(np.round(np.log1p(soft / c) * (1.0 / SQ)), 0, 15) \
          .astype(np.uint8)
    # (BC, T, K) -> (NCH, K, CHUNK, BC), nibble-packed along adjacent steps
    qt = q.reshape(BC, NCH, CHUNK, K).transpose(1, 3, 2, 0)
    packed = (qt[:, :, 0::2, :] | (qt[:, :, 1::2, :] << 4)) \
        .reshape(NCH, K, (CHUNK // 2) * BC)
    qconsts = np.zeros((K, 3), np.float32)
    qconsts[:, 0] = SQ
    qconsts[:, 1] = np.log(c)
    qconsts[:, 2] = c
    return np.ascontiguousarray(packed), qconsts, shift


def _make_consts(transitions):
    E = np.exp(transitions.astype(np.float32))
    ehat = np.zeros((K, 65), np.float32)
    ehat[:, 0:K] = E.T          # lhsT[j, i] = E[i, j]
    ehat[:, 64] = E[STOP, :]    # stop-dot capture column (D)
    pinit = np.zeros((K, K), np.float32)
    pinit[START, :] = 1.0
    return np.concatenate([ehat, pinit], axis=1).astype(ml_dtypes.bfloat16)


def _make_mask(lengths_core):
    """One-hot selection mask over D slots: hot at s = len_b for column b."""
    m = np.zeros((NWC * WCHUNK, BC), np.float32)
    m[lengths_core.astype(np.int64), np.arange(BC)] = 1.0
    return m.reshape(1, NWC * WCHUNK * 64).astype(NP_FP8)


def _postprocess(wout, shift, lengths_core):
    # wout: [1, WCHUNK*BC] — per (s % WCHUNK, b) partial sums, exactly one
    # nonzero per column b (the selected stop-dot D_{len_b})
    A = np.asarray(wout).astype(np.float64).reshape(WCHUNK, BC)
    D_sel = A.sum(axis=0)
    shift_cum = np.concatenate([np.zeros((1, BC)), np.cumsum(shift, axis=0)], axis=0)
    idx = lengths_core.astype(np.int64)
    return np.log(np.maximum(D_sel, 1e-300)) + shift_cum[idx, np.arange(BC)]


def _gold_score(feats, transitions, tags, lengths):
    Bb, Tt, _ = feats.shape
    t_idx = np.arange(Tt + 1)
    tags = tags.astype(np.int64)
    lengths = lengths.astype(np.int64)
    pad_start = np.concatenate([np.full((Bb, 1), START, tags.dtype), tags], axis=1)
    pad_stop = np.concatenate([tags, np.full((Bb, 1), STOP, tags.dtype)], axis=1)
    pad_stop = np.where(t_idx[None, :] >= lengths[:, None], STOP, pad_stop)
    trans_mask = (t_idx[None, :] <= lengths[:, None]).astype(np.float64)
    trans_score = np.sum(transitions[pad_stop, pad_start].astype(np.float64) * trans_mask, axis=1)
    emit_mask = (np.arange(Tt)[None, :] < lengths[:, None]).astype(np.float64)
    emit = np.take_along_axis(feats, tags[:, :, None], axis=2)[:, :, 0].astype(np.float64)
    emit_score = np.sum(emit * emit_mask, axis=1)
    return trans_score + emit_score


_NC_CACHE = {}


def _get_nc():
    if "nc" not in _NC_CACHE:
        _NC_CACHE["nc"] = _build_nc()
    return _NC_CACHE["nc"]


def kernel(feats, transitions, tags, lengths, _trace=False, _return_extra=False):
    feats = np.asarray(feats)
    transitions = np.asarray(transitions)
    tags = np.asarray(tags)
    lengths = np.asarray(lengths)

    consts = _make_consts(transitions)
    in_maps = []
    shifts = []
    for c in range(NCORES):
        fexp, qconsts, shift = _prep_core_inputs(feats[c * BC:(c + 1) * BC],
                                                 transitions)
        shifts.append(shift)
        mask = _make_mask(lengths[c * BC:(c + 1) * BC])
        in_maps.append({"consts": consts, "qconsts": qconsts, "fexp": fexp,
                        "mask": mask})

    _t0 = _time.time()
    res = run_bass_kernel_spmd(_get_nc(), in_maps, core_ids=list(range(NCORES)),
                               trace=_trace)
    _dev_s = _time.time() - _t0

    fwd = np.zeros((B,), np.float64)
    for c in range(NCORES):
        wout = np.asarray(res.results[c]["wout"])
        fwd[c * BC:(c + 1) * BC] = _postprocess(wout, shifts[c],
                                                lengths[c * BC:(c + 1) * BC])

    gold = _gold_score(feats, transitions, tags, lengths)
    loss = np.float32(np.mean(fwd - gold))
    out = np.array(loss, dtype=np.float32)
    if _return_extra:
        return out, {"fwd": fwd, "gold": gold, "exec_time_ns": res.exec_time_ns,
                     "device_call_s": _dev_s}
    return out
